# revision 50
# baseline (speedup 1.0000x reference)
"""Multi-head self-attention (RoPE, causal) Trainium2 Bass kernel.

Problem: B=4, S=2048, D=1024, H=16 heads, hd=64, fused QKV + RoPE +
causal softmax attention + output projection (torch-Linear convention).

Sharding: Megatron-style tensor parallel over heads. Each of the 8
NeuronCores owns 2 heads: it projects the full token stream through its
128-row slices of Wq/Wk/Wv, applies RoPE, runs causal attention for its
2 heads x 4 batches, and computes a partial output projection
h_core @ Wo[:, core_slice].T  (transposed layout). The host sums the 8
partial outputs and adds the output bias.

Per-core device pipeline (all activations SBUF-resident between phases):
  0. cos/sin tables for RoPE built on-device from the positions input
     (range-reduced Sin on ScalarE; elementwise prep on GpSimd).
  1. QKV projection. x is fed pre-transposed (xT[f_in, tok]); q,k are
     produced in [d, tok] layout, v in [tok, d] layout (v-aug with a
     ones column for softmax sums). RoPE = qA*cos + (R@qA)*sin with the
     pair-rotation R applied on the PE.
  2. Per (batch, head): scores computed transposed, sT[k,q] = kT.T@qT
     (contraction over d on partitions); exp on ScalarE with the 1/8
     scale fused; causal handled by skipping k>q blocks + masking the
     diagonal block on GpSimd. P_T stored triangular-packed in SBUF.
     Then out[q, d+1] = P_T.T @ [V | 1] accumulated over k tiles on the
     PE (softmax denominators ride along in column 64), in groups of 4
     q-tiles per PSUM bank; one strided reciprocal + one broadcast
     multiply per group normalizes and lands rows in hqb; paired-head
     [128,128] DMA transposes (issued on the Sync queue) build hT.
  3. Output projection, transposed: partial.T[f, tok] = WoT.T @ hT,
     PSUM evicted to a per-(batch,ft) staging row, one 2048-wide DMA.
Engine budget: PE does all matmuls; ACT does exp/sin; DVE does PSUM
post-processing; GpSimd does SBUF-only elementwise; SP issues DMAs.
"""

import os
import sys

for _p in ("/opt/trn_rl_repo",):
    if os.path.isdir(_p) and _p not in sys.path:
        sys.path.append(_p)

import math

import ml_dtypes
import numpy as np

import concourse.bass as bass
import concourse.mybir as mybir
import concourse.tile as tile
from concourse import bacc
from concourse.bass import ts, ds
from concourse.bass_utils import run_bass_kernel_spmd

BF16 = ml_dtypes.bfloat16

B = 4
S = 2048
D = 1024
H = 16
HD = 64
NCORES = 8
HPC = H // NCORES          # heads per core = 2
PC = HPC * HD              # partition rows per core's heads = 128
T = B * S                  # 8192 tokens
KT = D // 128              # f_in k-tiles = 8
NTOK = T // 128            # 64 token tiles of 128
SCALE = 1.0 / math.sqrt(HD)
ROPE_THETA = 10000.0

NQT = S // 128             # 16 q/k tiles per sequence
# triangular packing offsets for P_T: row kt covers q in [kt*128, S)
OFFS = [0] * NQT
for _kt in range(1, NQT):
    OFFS[_kt] = OFFS[_kt - 1] + (S - (_kt - 1) * 128)
PTRI_W = OFFS[-1] + (S - (NQT - 1) * 128)   # 17408


def _build_nc():
    nc = bacc.Bacc("TRN2", target_bir_lowering=False, debug=False,
                   num_devices=NCORES)
    dt = mybir.dt

    # ---- I/O ----
    x_in = nc.dram_tensor("x", [D, T], dt.bfloat16, kind="ExternalInput")
    cos_in = nc.dram_tensor("cosT", [128, S], dt.bfloat16, kind="ExternalInput")
    sin_in = nc.dram_tensor("sinT", [128, S], dt.bfloat16, kind="ExternalInput")
    wq_in = nc.dram_tensor("wq", [D, PC], dt.bfloat16, kind="ExternalInput")
    wk_in = nc.dram_tensor("wk", [D, PC], dt.bfloat16, kind="ExternalInput")
    wv_in = nc.dram_tensor("wv", [D, PC], dt.bfloat16, kind="ExternalInput")
    wo_in = nc.dram_tensor("wo", [PC, D], dt.bfloat16, kind="ExternalInput")
    bq_in = nc.dram_tensor("bq", [PC], dt.float32, kind="ExternalInput")
    bk_in = nc.dram_tensor("bk", [PC], dt.float32, kind="ExternalInput")
    bv_in = nc.dram_tensor("bv", [PC], dt.float32, kind="ExternalInput")
    out_d = nc.dram_tensor("out", [D, T], dt.bfloat16, kind="ExternalOutput")

    # ---- inline constants ----
    # RT = R.T where (R @ q)[2i] = -q[2i+1], (R @ q)[2i+1] = q[2i],
    # block-diagonal over the 2 stacked heads (structure identical).
    r = np.zeros((PC, PC), dtype=np.float32)
    for h in range(HPC):
        for i in range(HD // 2):
            r[h * HD + 2 * i, h * HD + 2 * i + 1] = -1.0
            r[h * HD + 2 * i + 1, h * HD + 2 * i] = 1.0
    rt_np = np.ascontiguousarray(r.T).astype(BF16)
    # causal mask for diagonal scoresT blocks: keep k_local <= q_local
    mask_np = np.tril(np.ones((128, 128), dtype=np.float32)).T.astype(BF16)

    rt_d = nc.inline_tensor(rt_np, "rt_c")
    mask_d = nc.inline_tensor(mask_np, "mask_c")
    id_d = nc.inline_tensor(np.eye(128, dtype=np.float32).astype(BF16), "id_c")

    fp32 = dt.float32
    bf16 = dt.bfloat16

    with tile.TileContext(nc) as tc:
        with (
            tc.tile_pool(name="consts", bufs=1) as consts,
            tc.tile_pool(name="resid", bufs=1) as resid,
            tc.tile_pool(name="xp", bufs=2) as xp,
            tc.tile_pool(name="work", bufs=3) as work,
            tc.tile_pool(name="ost", bufs=3) as ost,
            tc.tile_pool(name="ptri", bufs=2) as ptri_pool,
            tc.tile_pool(name="bigps", bufs=2, space="PSUM") as bigps,
            tc.tile_pool(name="accps", bufs=3, space="PSUM") as accps,
            tc.tile_pool(name="pvps", bufs=1, space="PSUM") as pvps,
        ):
            # ---- load constants / weights to SBUF ----
            # Issue order matters: SP dispatches DMAs serially (~0.6us
            # each) and the first q-projection matmul waits on wq + the
            # first x chunk, so those go first.
            wq_sb = consts.tile([128, KT, PC], bf16, tag="wq")
            wk_sb = consts.tile([128, KT, PC], bf16, tag="wk")
            wv_sb = consts.tile([128, KT, PC], bf16, tag="wv")
            xt0 = xp.tile([128, KT, 512], bf16, tag="xt")
            xTr0 = x_in.ap().rearrange("(kt p) n -> p kt n", p=128)
            # first weights + x chunk split fine so the first projection
            # matmuls start as soon as possible
            wqr = wq_in.ap().rearrange("(kt p) m -> p kt m", p=128)
            nc.sync.dma_start(out=wq_sb[:, 0:2, :], in_=wqr[:, 0:2, :])
            nc.sync.dma_start(out=xt0[:, 0:2, :], in_=xTr0[:, 0:2, 0:512])
            nc.sync.dma_start(out=wq_sb[:, 2:8, :], in_=wqr[:, 2:8, :])
            nc.sync.dma_start(out=xt0[:, 2:4, :], in_=xTr0[:, 2:4, 0:512])
            nc.sync.dma_start(out=xt0[:, 4:6, :], in_=xTr0[:, 4:6, 0:512])
            nc.sync.dma_start(out=xt0[:, 6:8, :], in_=xTr0[:, 6:8, 0:512])
            nc.sync.dma_start(
                out=wk_sb, in_=wk_in.ap().rearrange("(kt p) m -> p kt m", p=128))
            bq_sb = consts.tile([128, 1], fp32, tag="bq")
            nc.sync.dma_start(out=bq_sb, in_=bq_in.ap().rearrange("(p o) -> p o", o=1))
            bk_sb = consts.tile([128, 1], fp32, tag="bk")
            nc.sync.dma_start(out=bk_sb, in_=bk_in.ap().rearrange("(p o) -> p o", o=1))
            rt_sb = consts.tile([128, 128], bf16, tag="rt")
            nc.sync.dma_start(out=rt_sb, in_=rt_d[:, :])
            # RoPE cos/sin tables (host-computed from positions)
            cos_sb = consts.tile([128, S], bf16, tag="cosT")
            nc.sync.dma_start(out=cos_sb, in_=cos_in[:, :])
            sin_sb = consts.tile([128, S], bf16, tag="sinT")
            nc.sync.dma_start(out=sin_sb, in_=sin_in[:, :])
            nc.sync.dma_start(
                out=wv_sb, in_=wv_in.ap().rearrange("(kt p) m -> p kt m", p=128))
            # v bias broadcast over token partitions: [128, 128] f32
            bvb_sb = consts.tile([128, PC], fp32, tag="bvb")
            nc.sync.dma_start(
                out=bvb_sb,
                in_=bass.AP(tensor=bv_in, offset=0, ap=[[0, 128], [1, PC]]))
            mask_sb = consts.tile([128, 128], bf16, tag="mask")
            nc.sync.dma_start(out=mask_sb, in_=mask_d[:, :])
            wo_sb = consts.tile([128, D], bf16, tag="wo")
            nc.sync.dma_start(out=wo_sb, in_=wo_in[:, :])
            id_sb = consts.tile([128, 128], bf16, tag="ident")
            nc.sync.dma_start(out=id_sb, in_=id_d[:, :])

            # ---- residents ----
            qT = resid.tile([128, T], bf16, tag="qT")     # [d(2h), tok]
            kT = resid.tile([128, T], bf16, tag="kT")
            hT = resid.tile([128, T], bf16, tag="hT")
            # v natural + ones column: [tok%128, tok//128, head, 65]
            vA = resid.tile([128, NTOK, HPC, HD + 1], bf16, tag="vA")
            # normalized P@V outputs, paired across heads for one-shot
            # [128,128] DMA transposes into hT: [q%128, qt, head, d]
            hqb = resid.tile([128, NQT, HPC, HD], bf16, tag="hqb")
            nc.vector.memset(vA[:, :, :, HD:HD + 1], 1.0)

            # ---- phase 1: QKV projection + RoPE (emitted per token
            # chunk; interleaved with phase 2 below) ----
            TC = 512
            NTC = T // TC
            xTr = x_in.ap().rearrange("(kt p) n -> p kt n", p=128)
            xt_pref = {0: xt0}         # chunk 0 loaded with the consts

            def phase_1_load(tci):
                """Prefetch one x chunk; issued a few filler slots ahead
                of phase_1 so the ~4us DMA never stalls the PE."""
                xt = xp.tile([128, KT, TC], bf16, tag="xt")
                nc.sync.dma_start(out=xt, in_=xTr[:, :, ts(tci, TC)])
                xt_pref[tci] = xt

            def phase_1(tci):
                phase_1_qk(tci)
                phase_1_v(tci)

            def phase_1_qk(tci):
                tsl = ts(tci, TC)
                ssl = ds((tci * TC) % S, TC)
                xt = xt_pref[tci]
                # q/k projection accumulations first, then the rope
                # rotation matmuls, so the PE never waits on the DVE
                # bias-add between them.
                pa = {}
                ab = {}
                for which, w_sb, b_sb in (("q", wq_sb, bq_sb),
                                          ("k", wk_sb, bk_sb)):
                    p = accps.tile([128, TC], fp32, tag="acc")
                    for kt in range(KT):
                        nc.tensor.matmul(p, lhsT=w_sb[:, kt, :],
                                         rhs=xt[:, kt, :],
                                         start=(kt == 0), stop=(kt == KT - 1))
                    a_sb = work.tile([128, TC], bf16, tag="a_" + which,
                                     bufs=2)
                    nc.vector.tensor_scalar_add(a_sb, p, b_sb)
                    pa[which] = p
                    ab[which] = a_sb
                for which, dest in (("q", qT), ("k", kT)):
                    a_sb = ab[which]
                    pb = accps.tile([128, TC], fp32, tag="acc")
                    nc.tensor.matmul(pb, lhsT=rt_sb, rhs=a_sb,
                                     start=True, stop=True)
                    # rot = a*cos + b*sin  (tables indexed by s = tok % S)
                    t1 = work.tile([128, TC], bf16, tag="t1", bufs=2)
                    nc.gpsimd.tensor_mul(t1, a_sb, cos_sb[:, ssl])
                    t2 = work.tile([128, TC], bf16, tag="t2", bufs=2)
                    nc.vector.tensor_mul(t2, pb, sin_sb[:, ssl])
                    nc.gpsimd.tensor_add(dest[:, tsl], t1, t2)

            def phase_1_v(tci):
                xt = xt_pref.pop(tci)
                # v: natural layout, stationary = xT tiles
                for sub in range(TC // 128):
                    tt = tci * (TC // 128) + sub
                    pv = accps.tile([128, PC], fp32, tag="acc")
                    for kt in range(KT):
                        nc.tensor.matmul(
                            pv, lhsT=xt[:, kt, ds(sub * 128, 128)],
                            rhs=wv_sb[:, kt, :],
                            start=(kt == 0), stop=(kt == KT - 1))
                    nc.vector.tensor_add(
                        vA[:, tt, :, 0:HD],
                        pv.rearrange("p (h d) -> p h d", h=HPC),
                        bvb_sb.rearrange("p (h d) -> p h d", h=HPC))

            # ---- phase 2 + 3, software-pipelined over the 8 (b, h)
            # problems: emit scores/exp for problem i+1 interleaved with
            # P@V of problem i so the PE never drains while ACT runs exp.
            def phase_a(b, h, fillers=(), self_bgs=()):
                """scoresT + exp for one (batch, head) -> P_tri tile.
                fillers: independent emission closures (projection chunks,
                O-proj blocks, P@V groups of the previous problem) dropped
                in between score chunks so the PE always has scheduled
                work while ACT runs exp. self_bgs: (kt_threshold, closure)
                P@V groups of THIS problem, emitted as soon as the score
                rows they need exist (used for the last problem so the
                tail stays overlapped)."""
                fillers = list(fillers)
                self_bgs = list(self_bgs)
                base = b * S
                hsl = ds(h * HD, HD)
                pt = ptri_pool.tile([128, PTRI_W], bf16, tag="pt")
                # pack consecutive k-tile rows into shared psum tiles of
                # up to 1024 columns to amortize ACT instruction overhead
                kt = 0
                while kt < NQT:
                    group = [kt]
                    gw = S - kt * 128
                    while (gw < 1024 and group[-1] + 1 < NQT
                           and gw + (S - (group[-1] + 1) * 128) <= 1024):
                        group.append(group[-1] + 1)
                        gw += S - group[-1] * 128
                    if gw <= 1024:
                        sc = bigps.tile([128, 1024], fp32, tag="big")
                        o = 0
                        for g in group:
                            w = S - g * 128
                            kblk = kT[hsl, ds(base + g * 128, 128)]
                            for s0 in range(0, w, 512):
                                sw = min(512, w - s0)
                                nc.tensor.matmul(
                                    sc[:, ds(o + s0, sw)], lhsT=kblk,
                                    rhs=qT[hsl, ds(base + g * 128 + s0, sw)],
                                    start=True, stop=True)
                            o += w
                        nc.scalar.activation(
                            pt[:, ds(OFFS[group[0]], gw)], sc[:, 0:gw],
                            mybir.ActivationFunctionType.Exp, scale=SCALE)
                    else:
                        # single k-tile row wider than 1024: chunks
                        g = group[0]
                        w = S - g * 128
                        kblk = kT[hsl, ds(base + g * 128, 128)]
                        for c0 in range(0, w, 1024):
                            w2 = min(1024, w - c0)
                            sc = bigps.tile([128, 1024], fp32, tag="big")
                            for s0 in range(0, w2, 512):
                                sw = min(512, w2 - s0)
                                nc.tensor.matmul(
                                    sc[:, ds(s0, sw)], lhsT=kblk,
                                    rhs=qT[hsl, ds(base + g * 128 + c0 + s0, sw)],
                                    start=True, stop=True)
                            nc.scalar.activation(
                                pt[:, ds(OFFS[g] + c0, w2)], sc[:, 0:w2],
                                mybir.ActivationFunctionType.Exp, scale=SCALE)
                    for g in group:
                        # mask diagonal block (k_local > q_local -> 0)
                        dsl = ds(OFFS[g], 128)
                        nc.gpsimd.tensor_mul(pt[:, dsl], pt[:, dsl], mask_sb)
                    # filler first: its PE work covers the exp/mask
                    # latency the self P@V group is about to wait on
                    if fillers:
                        fillers.pop(0)()
                    while self_bgs and self_bgs[0][0] <= group[-1]:
                        self_bgs.pop(0)[1](pt)
                    kt = group[-1] + 1
                for f in fillers:
                    f()
                for _, f in self_bgs:
                    f(pt)
                return pt

            QG = 4                      # q-tiles per P@V psum group

            def phase_b_group(b, h, pt, g):
                """P@V for q-tiles [4g, 4g+4): accumulate over k tiles
                with P stationary into one PSUM bank; one strided
                reciprocal + one broadcast multiply normalizes all 4."""
                base = b * S
                acc = pvps.tile([128, QG * (HD + 1)], fp32, tag="pv")
                for j in range(QG):
                    qt = QG * g + j
                    for kt in range(qt + 1):
                        nc.tensor.matmul(
                            acc[:, ds(j * (HD + 1), HD + 1)],
                            lhsT=pt[:, ds(OFFS[kt] + (qt - kt) * 128, 128)],
                            rhs=vA[:, b * NQT + kt, h, :],
                            start=(kt == 0), stop=(kt == qt))
                acc3 = acc.rearrange("p (q c) -> p q c", c=HD + 1)
                rec = work.tile([128, QG], fp32, tag="rec", bufs=2)
                nc.vector.reciprocal(rec, acc3[:, :, HD:HD + 1])
                recb = bass.AP(tensor=rec.tensor, offset=rec.offset,
                               ap=[rec.ap[0], rec.ap[1], [0, HD]])
                nc.vector.tensor_mul(
                    hqb[:, ds(QG * g, QG), h, :], acc3[:, :, 0:HD], recb)

            def pe_transpose_group(b, g):
                """hqb -> hT transposes for q-tiles [4g, 4g+4) on the PE
                (DMA-transpose costs ~1.2us of sequencer issue each and
                starves the Sync queue): PE transpose into PSUM, DVE
                copy out. Scheduled one filler slot after the h==1 P@V
                group that writes hqb, so the PE queue never head-blocks
                on the hqb normalization."""
                base = b * S
                for j in range(QG):
                    qt = QG * g + j
                    tp = bigps.tile([128, 128], bf16, tag="big")
                    nc.tensor.transpose(tp, hqb[:, qt, :, :], id_sb)
                    nc.vector.tensor_copy(hT[:, ds(base + qt * 128, 128)], tp)

            NF = D // 128
            NFH = NF // 2
            oTr = out_d.ap().rearrange("(f p) t -> p f t", p=128)

            def phase_3(b, cc, half):
                """O-proj for one 512-token window x 4 output-feature
                tiles (half the features), staged then stored with one
                0.5MB DMA. Half-sized units interleave finely enough
                that the PE never waits on the PSUM-eviction chain."""
                tok = ds(b * S + cc * 512, 512)
                ostage = ost.tile([128, NFH, 512], bf16, tag="ostage")
                for fi in range(NFH):
                    ft = half * NFH + fi
                    po = accps.tile([128, 512], fp32, tag="acc")
                    nc.tensor.matmul(
                        po, lhsT=wo_sb[:, ts(ft, 128)], rhs=hT[:, tok],
                        start=True, stop=True)
                    if fi % 2 == 0:
                        nc.vector.tensor_copy(ostage[:, fi, :], po)
                    else:
                        nc.scalar.copy(ostage[:, fi, :], po)
                nc.sync.dma_start(
                    out=oTr[:, ds(half * NFH, NFH), tok], in_=ostage)

            # Explicit interleaved schedule. Notation: LD/CMP = x-chunk
            # prefetch / QKV+RoPE compute (prefetch leads its compute by
            # >= 2 slots so the ~4us DMA is hidden); BG(i,g) = P@V group
            # g of problem i (runs during phase_a(i+1)); P3(b,c) = O-proj
            # token-window. Work is balanced so every phase_a carries
            # more PE filler than its ~18us of ACT exp. The last
            # problem's P@V groups run inside its own phase_a via
            # self_bgs; only P3(3,*) remains for the tail.
            CPB = (S // TC)            # token chunks per batch = 4
            probs = [(b, h) for b in range(B) for h in range(HPC)]
            pts = {}

            def LD(b, c):
                return lambda: phase_1_load(b * CPB + c)

            def CMPa(b, c):
                return lambda: phase_1_qk(b * CPB + c)

            def CMPb(b, c):
                return lambda: phase_1_v(b * CPB + c)

            def P3(b, c, hf):
                return lambda: phase_3(b, c, hf)

            def TP(b, g):
                return lambda: pe_transpose_group(b, g)

            # batch 0 projection up front (xp ring depth is 3; chunk 0
            # was loaded with the consts)
            phase_1_load(1)
            phase_1_load(2)
            phase_1(0)
            phase_1_load(3)
            for cc in range(1, CPB):
                phase_1(cc)
            phase_1_load(CPB + 0)
            phase_1_load(CPB + 1)

            def BG(i, g):
                return lambda: phase_b_group(*probs[i], pts[i], g)

            FILL = {
                0: [CMPa(1, 0), LD(1, 2), CMPb(1, 0), CMPa(1, 1),
                    LD(1, 3), CMPb(1, 1), CMPa(1, 2), CMPb(1, 2),
                    CMPa(1, 3), CMPb(1, 3), LD(2, 0), LD(2, 1)],
                1: [BG(0, 0), CMPa(2, 0), BG(0, 1), CMPb(2, 0),
                    LD(2, 2), BG(0, 2), CMPa(2, 1), BG(0, 3),
                    CMPb(2, 1), LD(2, 3)],
                2: [BG(1, 0), CMPa(2, 2), TP(0, 0), BG(1, 1),
                    CMPb(2, 2), TP(0, 1), LD(3, 0), BG(1, 2),
                    CMPa(2, 3), TP(0, 2), CMPb(2, 3), LD(3, 1),
                    BG(1, 3), TP(0, 3)],
                3: [BG(2, 0), CMPa(3, 0), LD(3, 2), BG(2, 1),
                    CMPb(3, 0), P3(0, 0, 0), BG(2, 2), CMPa(3, 1),
                    P3(0, 0, 1), BG(2, 3), CMPb(3, 1), LD(3, 3),
                    P3(0, 1, 0), P3(0, 1, 1)],
                4: [BG(3, 0), CMPa(3, 2), TP(1, 0), BG(3, 1),
                    CMPb(3, 2), TP(1, 1), BG(3, 2), CMPa(3, 3),
                    TP(1, 2), P3(0, 2, 0), BG(3, 3), CMPb(3, 3),
                    TP(1, 3), P3(0, 2, 1)],
                5: [BG(4, 0), P3(0, 3, 0), P3(1, 0, 0), BG(4, 1),
                    P3(0, 3, 1), P3(1, 0, 1), BG(4, 2), P3(1, 1, 0),
                    P3(1, 1, 1), BG(4, 3), P3(1, 2, 0), P3(1, 2, 1)],
                # problem 5 is (2,1): each P3(2,*) window trails the P@V
                # group + PE transpose that produce its hT slice
                6: [BG(5, 0), P3(1, 3, 0), BG(5, 1), TP(2, 0),
                    P3(1, 3, 1), P3(2, 0, 0), BG(5, 2), TP(2, 1),
                    P3(2, 0, 1), P3(2, 1, 0), BG(5, 3), TP(2, 2),
                    P3(2, 1, 1), P3(2, 2, 0), TP(2, 3), P3(2, 2, 1)],
                7: [BG(6, 0), P3(2, 3, 0), BG(6, 1), P3(2, 3, 1),
                    BG(6, 2), BG(6, 3)],
            }
            LAST = len(probs) - 1

            def SBG(g):
                return lambda pt: phase_b_group(*probs[LAST], pt, g)

            def TPG(g):
                return lambda pt: pe_transpose_group(B - 1, g)

            def SP3(c, hf):
                return lambda pt: phase_3(B - 1, c, hf)

            for i in range(len(probs)):
                self_bgs = []
                if i == LAST:
                    # last problem: its own P@V groups fire as soon as
                    # the score rows they need are exponentiated; hqb
                    # transposes run on the PE one boundary later; the
                    # first batch-3 O-proj windows chase them so only
                    # the final windows trail the loop
                    self_bgs = [
                        (3, SBG(0)), (4, TPG(0)),
                        (7, SBG(1)), (8, TPG(1)),
                        (8, SP3(0, 0)), (10, SP3(0, 1)),
                        (11, SBG(2)), (12, SP3(1, 0)),
                        (14, TPG(2)), (14, SP3(1, 1)),
                        (15, SBG(3)),
                    ]
                pts[i] = phase_a(*probs[i], fillers=FILL[i],
                                 self_bgs=self_bgs)
            phase_3(B - 1, 2, 0)
            pe_transpose_group(B - 1, 3)
            phase_3(B - 1, 2, 1)
            phase_3(B - 1, 3, 0)
            phase_3(B - 1, 3, 1)

    nc.compile()
    return nc


_NC_CACHE = None


def _get_nc():
    global _NC_CACHE
    if _NC_CACHE is None:
        _NC_CACHE = _build_nc()
    return _NC_CACHE


def kernel(x, positions, Wqkv, bqkv, Wo, bo):
    x = np.asarray(x)
    positions = np.asarray(positions)
    Wqkv = np.asarray(Wqkv)
    bqkv = np.asarray(bqkv)
    Wo = np.asarray(Wo)
    bo = np.asarray(bo)

    nc = _get_nc()

    xT = np.ascontiguousarray(x.reshape(T, D).T).astype(BF16)
    # RoPE cos/sin tables, host-computed: row p uses head-local pair
    # frequency (p % 64) // 2; all batches share one positions row.
    pos = np.asarray(positions[0], dtype=np.float64)
    inv = 1.0 / (ROPE_THETA ** (2.0 * ((np.arange(PC) % HD) // 2) / HD))
    ang = inv[:, None] * pos[None, :]
    cosT = np.ascontiguousarray(np.cos(ang)).astype(BF16)
    sinT = np.ascontiguousarray(np.sin(ang)).astype(BF16)

    in_maps = []
    for c in range(NCORES):
        r0 = c * PC
        wq = np.ascontiguousarray(Wqkv[r0:r0 + PC, :].T).astype(BF16)
        wk = np.ascontiguousarray(Wqkv[D + r0:D + r0 + PC, :].T).astype(BF16)
        wv = np.ascontiguousarray(Wqkv[2 * D + r0:2 * D + r0 + PC, :].T).astype(BF16)
        wo = np.ascontiguousarray(Wo[:, r0:r0 + PC].T).astype(BF16)
        in_maps.append({
            "x": xT, "cosT": cosT, "sinT": sinT,
            "wq": wq, "wk": wk, "wv": wv, "wo": wo,
            "bq": bqkv[r0:r0 + PC].astype(np.float32),
            "bk": bqkv[D + r0:D + r0 + PC].astype(np.float32),
            "bv": bqkv[2 * D + r0:2 * D + r0 + PC].astype(np.float32),
        })

    res = run_bass_kernel_spmd(nc, in_maps, core_ids=list(range(NCORES)))
    acc = res.results[0]["out"].astype(np.float32)
    for c in range(1, NCORES):
        acc += res.results[c]["out"].astype(np.float32)
    out = acc + bo[:, None].astype(np.float32)
    return np.ascontiguousarray(out.T).reshape(B, S, D)
